# revision 31
# baseline (speedup 1.0000x reference)
"""Trainium2 Bass kernel for nn_CNNRandomProjection (B=256, C=128, H=W=32).

Reference computation:
    y[b,c,k,w] = sum_h P[c,k,h] * x[b,c,h,w]
    y = lam * y ; y = leaky_relu(y, 0.2)
    out = gamma * (y - mean_c) * rsqrt(var_c + 1e-5) + beta     (stats over B,H,W)

Distribution: shard the CHANNEL axis across the 8 NeuronCores (16 channels
per core). BatchNorm statistics are per-channel, so each core owns the full
batch for its channels and no cross-core communication is needed.

Per core the 16 channels are processed as 4 groups of 4 channels. For each
group a 128x128 block-diagonal weight tile (4 diagonal 32x32 blocks, each
P[c].T) contracts 4 channels x 32 h at once:  psum[32i+k, col] =
sum_h W[32i+h, 32i+k] * x[32i+h, col], with col = (batch, w) packed along
the free dim.  ScalarE applies lam (runtime scale) + leaky-relu while moving
PSUM->SBUF; VectorE bn_stats accumulates per-partition mean/var in the same
pass; two tiny selector matmuls fold the stats across partitions and expand
the per-channel affine (a, b) back to partitions; a single fused
tensor_scalar (y*a + b) and a contiguous DMA store finish each tile.

The host packs x into the exact SBUF tile layout so every DMA is fully
contiguous (8 KB per partition per transfer).
"""

import numpy as np

import concourse.bass as bass
import concourse.bacc as bacc
import concourse.tile as tile
from concourse import mybir
from concourse.bass_utils import run_bass_kernel_spmd

# ---------------------------------------------------------------- constants
B, C, H, W = 256, 128, 32, 32
NCORES = 8
CLOC = C // NCORES          # channels per core = 16
BN_EPS = 1e-5
NEG_SLOPE = 0.2
F32 = mybir.dt.float32
# bf16 x/W for the projection matmul: the PE runs bf16 at 1 cycle/row vs 4
# for fp32 (78us -> 20us of PE time), and the fp32->bf16 cast rides the
# SWDGE load DMA for free, halving the load-side DMA-engine occupancy.
# Accumulation stays fp32 in PSUM; end-to-end rel err ~3e-3 vs the 2e-2
# harness tolerance.
BF16 = mybir.dt.bfloat16


class Cfg:
    """Geometry of the per-core kernel (parametrized so a mini version can
    run through the interpreter)."""

    def __init__(self, G=4, NJG=4, TS=2048):
        self.G = G                    # channel groups (4 channels each)
        self.NJG = NJG                # DMA tiles per group
        self.TS = TS                  # free-dim columns per tile
        self.NQ = TS // 512           # matmuls (512-col chunks) per tile
        self.NB = NJG * self.NQ * 16  # batches covered (16 batches per 512 cols)
        self.NFREE = NJG * TS         # free elements per partition per group
        self.NTOT = 32 * self.NFREE   # BN element count per channel (32 k-rows)


FULL = Cfg()
assert FULL.NB == B and FULL.G * 4 == CLOC


# ------------------------------------------------------------- bass program
def build_nc(cfg: Cfg, reps: int = 1, mode: str = "full"):
    G, NJG, TS, NQ = cfg.G, cfg.NJG, cfg.TS, cfg.NQ
    # Bacc (not raw Bass): its compile() runs generate_event_semaphores,
    # which legalizes to the TRN2 1-sync-wait-per-instruction constraint.
    # Bigger SWDGE descriptor ring (1536 descs = 12 in-flight 128-desc
    # loads) so load descriptor generation isn't ring-throttled to transfer
    # pace — the partition reduces queued behind the gens on the Pool
    # engine then run ~7us earlier.
    nc = bacc.Bacc("TRN2", target_bir_lowering=False, debug=False,
                   dynamic_dma_scratch_size=24576)

    xt = nc.dram_tensor("xt", [G, NJG, 128, TS], F32, kind="ExternalInput")
    wt = nc.dram_tensor("wt", [128, G * 128], BF16, kind="ExternalInput")
    ct = nc.dram_tensor("ct", [128, const_cols(cfg)], F32, kind="ExternalInput")
    # bf16 output: halves store-side HBM traffic (the kernel is HBM-bound;
    # 16.8 MB load + 8.4 MB store = 25.2 MB/core vs 33.6 fp32). The host
    # unpack upcasts to fp32; bf16 output quantization adds ~1e-3 rel err
    # against the 2e-2 harness tolerance.
    yt = nc.dram_tensor("yt", [G, NJG, 128, TS], BF16, kind="ExternalOutput")

    with tile.TileContext(nc) as tc:
        _body(tc, {"yt": yt.ap()},
              {"xt": xt.ap(), "wt": wt.ap(), "ct": ct.ap()},
              cfg, reps=reps, mode=mode)
    nc.compile()
    return nc


def _const_offsets(cfg: Cfg):
    """Column offsets inside the packed constants panel [128, NCOLS]:
    lam | zero | gb(per-partition expanded gamma/beta, 2G cols) | eps.
    (The block-diagonal weights travel separately as bf16.)"""
    G = cfg.G
    o = {}
    o["lam"] = 0
    o["zero"] = o["lam"] + 1
    o["gb"] = o["zero"] + 1
    o["eps"] = o["gb"] + 2 * G
    o["end"] = o["eps"] + 1
    return o


def const_cols(cfg: Cfg):
    return _const_offsets(cfg)["end"]


def _body(tc, outs, ins, cfg: Cfg, reps: int = 1, mode: str = "full"):
    """Kernel body over DRAM APs (shared by the HW path and the interp test).
    reps > 1 wraps the whole body in a hardware For_i loop — used only by the
    timing bench to amplify device time above the dispatch-noise floor.
    mode: "full" = real kernel; "dmaonly" = just the load + store streams
    (garbage output) to measure the DMA roofline of this access pattern."""
    nc = tc.nc
    G, NJG, TS, NQ = cfg.G, cfg.NJG, cfg.TS, cfg.NQ
    xt, wt, ct = ins["xt"], ins["wt"], ins["ct"]
    yt = outs["yt"]
    off = _const_offsets(cfg)

    from contextlib import ExitStack
    with ExitStack() as ctx:
        singles = ctx.enter_context(tc.tile_pool(name="singles", bufs=1))
        xpool = ctx.enter_context(tc.tile_pool(name="xp", bufs=16))
        ypool = ctx.enter_context(tc.tile_pool(name="yp", bufs=1))
        # bf16 staging for normalized output: 8 bufs so the tail's 8
        # back-to-back normalize+store pairs never serialize on a staging
        # WAR (bufs=3 paced the tail at one store per 1.6us).
        spool = ctx.enter_context(tc.tile_pool(name="st", bufs=8))

        # two-bank [128, 2, 512] psum tiles; 3 bufs (+1 absorber bank) = 7 of
        # the 8 banks. Each tile takes two matmuls (one per bank) and ONE
        # [128,1024] Prelu drain: ACT carries ~370ns fixed overhead per
        # Activation, so halving the instruction count cuts ACT busy from
        # ~56us to ~33us/rep.
        pspool = ctx.enter_context(tc.tile_pool(name="ps", bufs=3, space="PSUM"))
        # Scratch PSUM bank for "wait absorber" matmuls: walrus allows only a
        # single sync-wait on a Matmult (it lands on the LDWEIGHTS half), so
        # before each tile's real matmuls a dummy 1x1 matmul absorbs the
        # x-DMA semaphore wait into PE's vector clock; the real matmuls then
        # only ever carry the one PSUM-WAR wait.
        absp = ctx.enter_context(tc.tile_pool(name="absp", bufs=1, space="PSUM"))
        abs_ps = absp.tile([1, 1], F32, tag="abs", name="abs_ps")

        if mode in ("dmaonly", "loadonly"):
            if reps > 1:
                ctx.enter_context(tc.For_i(0, reps, 1, staggered_reset=True))
            src = singles.tile([128, TS], BF16, tag="dsrc", name="dsrc")
            nc.vector.memset(src[:, 0:1], 0.0)
            for g in range(G):
                for jg in range(NJG):
                    xtile = xpool.tile([128, TS], BF16, tag="x", name=f"dx_{g}_{jg}")
                    nc.gpsimd.dma_start(out=xtile, in_=xt[g, jg])
                    if mode == "dmaonly":
                        nc.sync.dma_start(out=yt[g, jg], in_=src)
            return

        # Small constant DMAs: the bf16 block-diag weights, then the fp32
        # panel (lam broadcast, a zero column, the two selector matrices,
        # gamma/beta and eps).
        w_sb = singles.tile([128, G * 128], BF16, name="w_sb")
        nc.sync.dma_start(out=w_sb, in_=wt)
        c_sb = singles.tile([128, off["end"]], F32)
        nc.sync.dma_start(out=c_sb, in_=ct)
        lam_sb = c_sb[:, off["lam"]:off["lam"] + 1]
        zero_sb = c_sb[:, off["zero"]:off["zero"] + 1]
        gb_sb = c_sb[:, off["gb"]:off["gb"] + 2 * G]
        eps_sb = c_sb[:, off["eps"]:off["eps"] + 1]
        # ACT warmup: observe the const-DMA semaphore once so the per-tile
        # Prelu activations only ever carry the single PE sync-wait.
        act_warm = singles.tile([128, 1], F32)
        nc.scalar.activation(out=act_warm, in_=zero_sb,
                             func=mybir.ActivationFunctionType.Identity,
                             bias=zero_sb, scale=1.0)
        # PE warmup: observe the W-DMA semaphore once so the per-tile real
        # matmuls never need a second wait.
        nc.tensor.matmul(abs_ps, w_sb[0:1, 0:1], w_sb[0:1, 0:1],
                         start=True, stop=True)

        # bn_stats is HW-limited to 512 free elements, so one entry per
        # 512-col chunk (BNStats also has no 16-bit DVE fast mode).
        stats = singles.tile([128, G, NJG * NQ, 6], F32)

        # x loads ride the Pool/SWDGE queue. With bufs=16 every tile has its
        # own buffer (x is fully SBUF-resident as bf16), so no load ever
        # waits on compute and the load half of the DMA stream runs
        # back-to-back from t~2us. Only the first 3 groups' loads go
        # up-front: the SWDGE descriptor ring holds ~8 DMAs, so generation
        # runs at transfer pace and the in-order Pool queue would otherwise
        # park group 0's partition_all_reduce behind all 16 gens (~26us).
        # The last group's loads are emitted after g0's fold stage 0 instead.
        xtiles = {}
        ytiles = {}

        def load_x(g, jg):
            # gpsimd (SWDGE) so the DMA itself casts fp32 -> bf16: halves
            # the load-side DMA-engine occupancy and feeds the PE its bf16
            # operand with no extra compute pass.
            xtile = xpool.tile([128, TS], BF16, tag="x", name=f"x_{g}_{jg}")
            nc.gpsimd.dma_start(out=xtile, in_=xt[g, jg])
            xtiles[g, jg] = xtile

        # Per-group stats fold, cut into stages that are emitted interleaved
        # with the NEXT group's tile blocks (software pipeline). Every engine
        # queue is in-order, so each stage must reach the head of its queue
        # only after its inputs are ready. The fold never touches PE: the
        # across-partition reduction runs per 32-partition block on the
        # otherwise-idle Pool/GPSIMD engine, and the rest of the chain stays
        # 128 partitions wide (gamma/beta arrive host-expanded per
        # partition), so no expansion matmul is needed. Nothing ever
        # head-of-line blocks a saturated engine.
        fs = {}
        # si tiles are [128, 32] so the 32x32 stream transpose can fold
        # them; cols 2..31 are zeroed once so the transpose/reduce of the
        # unused columns never sees uninitialized SBUF.
        for f0 in range(G):
            si0 = singles.tile([128, 32], F32, tag=f"si{f0}", name=f"si_{f0}")
            nc.vector.memset(si0, 0.0)
            fs[f0] = {"si": si0}

        # The timing loop starts here: constants, warmups and memsets above
        # are genuinely one-time costs (in the single-shot kernel they hide
        # under the first load's DGE latency), so the slope bench measures
        # the steady-state pipeline only.
        # unroll=1: measured on HW, two body copies per iteration pipelined
        # WORSE than the staggered-reset back-edge (109us vs 77us per rep) —
        # the scheduler overlaps consecutive loop iterations better than
        # intra-body copies.
        unroll = int(mode[6:]) if mode.startswith("unroll") else 1
        if reps > 1:
            assert reps % unroll == 0
            ctx.enter_context(tc.For_i(0, reps // unroll, 1,
                                       staggered_reset=True))

        def emit_once():
            for g in range(G - 1):
                for jg in range(NJG):
                    load_x(g, jg)
            emit_main()

        def fold_stage(f, stage):
            if stage == 0:
                # per-partition mean/var -> scaled sum & sum-of-squares
                # (emitted right after group f's own bn_stats: DVE reaches
                # these with data already in hand); then fold across each
                # channel's 32 partitions entirely on DVE via two 32x32
                # stream transposes: transpose + free-dim reduce puts each
                # channel's (S, SS) on partitions 32c+{0,1}; broadcast-copy
                # + transpose replicates them to every partition of the
                # block. (gpsimd.partition_all_reduce silently corrupts
                # base-partition!=0 blocks on hardware.)
                mv = singles.tile([128, 2], F32, tag=f"mv{f}", name=f"mv_{f}")
                nc.vector.bn_aggr(out=mv, in_=stats[:, f, :, :])
                si = fs[f]["si"]
                nc.vector.tensor_copy(si[:, 0:1], mv[:, 0:1])
                nc.vector.tensor_mul(si[:, 1:2], mv[:, 0:1], mv[:, 0:1])
                nc.vector.tensor_add(si[:, 1:2], si[:, 1:2], mv[:, 1:2])
                nc.vector.tensor_scalar_mul(si[:, 0:2], si[:, 0:2],
                                            float(cfg.NFREE))
                T = singles.tile([128, 32], F32, tag=f"T{f}", name=f"T_{f}")
                nc.vector.transpose(T, si)
                red = singles.tile([128, 1], F32, tag=f"red{f}", name=f"red_{f}")
                nc.vector.reduce_sum(red, T, axis=mybir.AxisListType.X)
                U = singles.tile([128, 32], F32, tag=f"U{f}", name=f"U_{f}")
                nc.vector.tensor_copy(U, red[:, 0:1].to_broadcast((128, 32)))
                V = singles.tile([128, 32], F32, tag=f"V{f}", name=f"V_{f}")
                nc.vector.transpose(V, U)
                fs[f]["sAR"] = V[:, 0:2]
            elif stage == 1:
                # per-partition scalar math on DVE — every partition holds
                # its own channel's stats, so everything stays 128 wide.
                chan = singles.tile([128, 2], F32, tag=f"chan{f}",
                                    name=f"chan_{f}")
                nc.vector.tensor_scalar_mul(chan, fs[f]["sAR"],
                                            1.0 / float(cfg.NTOT))
                var1 = singles.tile([128, 1], F32, tag=f"var{f}", name=f"var_{f}")
                nc.vector.tensor_mul(var1, chan[:, 0:1], chan[:, 0:1])
                nc.vector.tensor_sub(var1, chan[:, 1:2], var1)
                fs[f]["chan"] = chan
                fs[f]["var1"] = var1
            elif stage == 2:
                # sqrt(var + eps) on ACT — same act table set as Prelu, so
                # no table reload.
                nc.scalar.activation(out=fs[f]["var1"], in_=fs[f]["var1"],
                                     func=mybir.ActivationFunctionType.Sqrt,
                                     bias=eps_sb[:, :], scale=1.0)
            elif stage == 3:
                chan, var1 = fs[f]["chan"], fs[f]["var1"]
                nc.vector.reciprocal(var1, var1)   # 1/sqrt(var+eps)
                ab = singles.tile([128, 2], F32, tag=f"ab{f}", name=f"ab_{f}")
                nc.vector.tensor_mul(ab[:, 0:1], gb_sb[:, f:f + 1], var1)
                nc.vector.tensor_mul(ab[:, 1:2], chan[:, 0:1], ab[:, 0:1])
                nc.vector.tensor_sub(ab[:, 1:2], gb_sb[:, G + f:G + f + 1],
                                     ab[:, 1:2])
                fs[f]["ab"] = ab
            else:
                raise AssertionError("stage 4 replaced by norm_store")

        def norm_store(f, jg):
            # normalize (bf16 -> bf16 on DVE: TensorScalarPtr with all-SBUF
            # 2-byte packed operands hits the 4x_2p perf mode, ~0.6us per
            # 2048-col tile) into a staging tile, then store.
            ab = fs[f]["ab"]
            ytile = ytiles[f, jg]
            stile = spool.tile([128, NQ, 512], BF16, tag="st",
                               name=f"st_{f}_{jg}")
            nc.vector.tensor_scalar(
                out=stile, in0=ytile,
                scalar1=ab[:, 0:1], scalar2=ab[:, 1:2],
                op0=mybir.AluOpType.mult, op1=mybir.AluOpType.add)
            # stores go out on the SP HWDGE queue: the casting loads own
            # GPSIMD's SWDGE queue, and a store waiting on this group's
            # normalize must not head-of-line-block loads.
            if mode != "nostore":
                nc.sync.dma_start(out=yt[f, jg], in_=stile)

        def emit_main():
            for g in range(G):
                for jg in range(NJG):
                    xtile = xtiles[g, jg]
                    # bf16 y tile: bn_stats runs at the DVE 16-bit 2x rate and
                    # SBUF stays light; the bf16 rounding happens before the
                    # batch stats, so stats and normalize see the same values.
                    ytile = ypool.tile([128, NQ, 512], BF16, tag=f"y_{g}_{jg}",
                                       name=f"y_{g}_{jg}")
                    ytiles[g, jg] = ytile
                    nc.tensor.matmul(abs_ps, xtile[0:1, 0:1], xtile[0:1, 0:1],
                                     start=True, stop=True)
                    for h in range(NQ // 2):
                        ps = pspool.tile([128, 2, 512], F32, tag="mm",
                                         name=f"mm_{g}_{jg}_{h}")
                        for j in range(2):
                            q = 2 * h + j
                            nc.tensor.matmul(ps[:, j, :],
                                             w_sb[:, g * 128:(g + 1) * 128],
                                             xtile[:, q * 512:(q + 1) * 512],
                                             start=True, stop=True)
                        # NOTE: Prelu, not Lrelu — the HW Lrelu table ignores the
                        # alpha operand (fixed 0.01 slope); Prelu honors it.
                        # lam is folded into the weights on the host (BN is
                        # scale-invariant up to the eps'=eps/lam^2 correction
                        # packed into the constants panel): scale=1.
                        nc.scalar.activation(
                            out=ytile[:, 2 * h:2 * h + 2, :], in_=ps,
                            func=mybir.ActivationFunctionType.Prelu,
                            bias=zero_sb[:, :], scale=1.0, alpha=NEG_SLOPE)
                        # bn_stats right behind each Prelu (HW caps bn_stats
                        # at 512 free elements, so two per 1024-col drain):
                        # keeps the last tile's stats chain short.
                        for j in range(2):
                            q = 2 * h + j
                            nc.vector.bn_stats(out=stats[:, g, jg * NQ + q, :],
                                               in_=ytile[:, q, :])
                    if mode == "nofold":
                        # diagnostic: store the un-normalized bf16 y directly,
                        # skipping the stats fold + normalize stages.
                        nc.sync.dma_start(out=yt[g, jg], in_=ytile)
                        if g == 1 and jg == NJG - 1:
                            for jg2 in range(NJG):
                                load_x(G - 1, jg2)
                        continue
                    if g >= 1:
                        if jg == 0:
                            # fold of the previous group right after this
                            # group's first tile: it clears the queues while
                            # this group's remaining tiles are still
                            # projecting.
                            for stage in range(1, 4):
                                fold_stage(g - 1, stage)
                        # one normalize+store per tile slot: spreads the
                        # store DMA traffic evenly through the rep instead of
                        # bursting 4 stores at each group boundary.
                        norm_store(g - 1, jg)
                    if jg == NJG - 1:
                        fold_stage(g, 0)
                        if g == 1:
                            # last group's loads, queued behind g0's AND g1's
                            # partition reduces on the Pool queue: both early
                            # folds clear the queue before these four gens, and
                            # the transfers backfill the DMA engines behind the
                            # first stores.
                            for jg2 in range(NJG):
                                load_x(G - 1, jg2)
            if mode != "nofold":
                for stage in range(1, 4):
                    fold_stage(G - 1, stage)
                for jg in range(NJG):
                    norm_store(G - 1, jg)

        for _u in range(unroll):
            emit_once()


# ------------------------------------------------------------ host packing
def _pack_x_shard(xs, cfg: Cfg):
    """xs [NB, 4G, 32, 32] -> [G, NJG, 128, TS] tile layout.
    partition = 32*i + h ; col = jj*512 + bl*32 + w ; b = jg*(NQ*16) + jj*16 + bl."""
    G, NJG, NQ, TS = cfg.G, cfg.NJG, cfg.NQ, cfg.TS
    t = xs.reshape(NJG, NQ, 16, G, 4, H, W)          # [jg, jj, bl, g, i, h, w]
    t = t.transpose(3, 0, 4, 5, 1, 2, 6)             # [g, jg, i, h, jj, bl, w]
    return np.ascontiguousarray(t).reshape(G, NJG, 128, TS)


def _unpack_y_shard(ytv, cfg: Cfg):
    """[G, NJG, 128, TS] -> [NB, 4G, 32, 32]."""
    G, NJG, NQ, TS = cfg.G, cfg.NJG, cfg.NQ, cfg.TS
    t = ytv.reshape(G, NJG, 4, 32, NQ, 16, W)        # [g, jg, i, k, jj, bl, w]
    t = t.transpose(1, 4, 5, 0, 2, 3, 6)             # [jg, jj, bl, g, i, k, w]
    return t.reshape(cfg.NB, 4 * G, H, W)


def _pack_w(Pshard, cfg: Cfg, sgn=1.0):
    """Block-diagonal bf16 weight panel [128, G*128]: per group g four
    diagonal 32x32 blocks, each sgn*P[4g+i].T. BatchNorm is invariant to a
    positive scale on its input, so only sign(lam) must reach the kernel —
    |lam| is folded away entirely and the device never sees lam."""
    import ml_dtypes
    G = cfg.G
    w = np.zeros((128, G * 128), np.float32)
    for g in range(G):
        for i in range(4):
            w[32 * i:32 * (i + 1),
              g * 128 + 32 * i:g * 128 + 32 * (i + 1)] = Pshard[4 * g + i].T
    return (np.float32(sgn) * w).astype(ml_dtypes.bfloat16)


def _pack_const(Pshard, lam, gamma_s, beta_s, cfg: Cfg):
    """Pack the small fp32 constants into one [128, NCOLS] panel.
    gamma/beta are pre-expanded per partition: col g holds
    gamma[4g + p//32] at partition p (the fold chain stays 128 wide)."""
    G = cfg.G
    off = _const_offsets(cfg)
    c = np.zeros((128, off["end"]), np.float32)
    c[:, off["lam"]] = np.float32(lam[0])
    # off["zero"] column stays 0
    blk = np.arange(128) // 32                      # channel-in-group index
    for g in range(G):
        c[:, off["gb"] + g] = gamma_s[4 * g + blk]
        c[:, off["gb"] + G + g] = beta_s[4 * g + blk]
    # The kernel computes stats of u = leaky(sign(lam)*proj), i.e. y/|lam|.
    # Exactly: (y-mean_y)*rsqrt(var_y+eps) == (u-mean_u)*rsqrt(var_u+eps/lam^2),
    # so the eps the kernel adds must be pre-divided by lam^2.
    lam2 = float(lam[0]) ** 2
    c[:, off["eps"]] = BN_EPS / lam2 if lam2 > 0 else BN_EPS
    return c


def make_in_maps(x, P, lam, gamma, beta, cfg: Cfg = FULL, ncores: int = NCORES):
    cl = 4 * cfg.G
    sgn = 1.0 if float(lam[0]) >= 0 else -1.0
    maps = []
    for m in range(ncores):
        sl = slice(m * cl, (m + 1) * cl)
        maps.append({
            "xt": _pack_x_shard(np.ascontiguousarray(x[:, sl]), cfg),
            "wt": _pack_w(P[sl], cfg, sgn),
            "ct": _pack_const(P[sl], lam, gamma[sl], beta[sl], cfg),
        })
    return maps


_NC_CACHE = {}


def _get_nc(cfg: Cfg = FULL):
    key = (cfg.G, cfg.NJG, cfg.TS)
    if key not in _NC_CACHE:
        _NC_CACHE[key] = build_nc(cfg)
    return _NC_CACHE[key]


def run(inputs, trace=False, tmpdir=None):
    """Run on the 8 NeuronCores; returns (out, BassKernelResults)."""
    x = np.asarray(inputs["x"], np.float32)
    P = np.asarray(inputs["P"], np.float32)
    lam = np.asarray(inputs["lam"], np.float32)
    gamma = np.asarray(inputs["gamma"], np.float32)
    beta = np.asarray(inputs["beta"], np.float32)

    if float(lam[0]) == 0.0:
        # y == 0 everywhere -> BN emits exactly beta (matches reference).
        out = np.broadcast_to(beta[None, :, None, None],
                              (B, C, H, W)).astype(np.float32).copy()
        return out, None

    nc = _get_nc(FULL)
    in_maps = make_in_maps(x, P, lam, gamma, beta, FULL)
    res = run_bass_kernel_spmd(nc, in_maps, core_ids=list(range(NCORES)),
                               trace=trace, tmpdir=tmpdir)
    out = np.empty((B, C, H, W), np.float32)
    for m in range(NCORES):
        out[:, m * CLOC:(m + 1) * CLOC] = _unpack_y_shard(
            np.asarray(res.results[m]["yt"]).astype(np.float32), FULL)
    return out, res


def kernel(**inputs):
    out, _ = run(inputs)
    return out



# revision 32
# speedup vs baseline: 1.0264x; 1.0264x over previous
"""Trainium2 Bass kernel for nn_CNNRandomProjection (B=256, C=128, H=W=32).

Reference computation:
    y[b,c,k,w] = sum_h P[c,k,h] * x[b,c,h,w]
    y = lam * y ; y = leaky_relu(y, 0.2)
    out = gamma * (y - mean_c) * rsqrt(var_c + 1e-5) + beta     (stats over B,H,W)

Distribution: shard the CHANNEL axis across the 8 NeuronCores (16 channels
per core). BatchNorm statistics are per-channel, so each core owns the full
batch for its channels and no cross-core communication is needed.

Per core the 16 channels are processed as 4 groups of 4 channels. For each
group a 128x128 block-diagonal weight tile (4 diagonal 32x32 blocks, each
P[c].T) contracts 4 channels x 32 h at once:  psum[32i+k, col] =
sum_h W[32i+h, 32i+k] * x[32i+h, col], with col = (batch, w) packed along
the free dim.  ScalarE applies lam (runtime scale) + leaky-relu while moving
PSUM->SBUF; VectorE bn_stats accumulates per-partition mean/var in the same
pass; two tiny selector matmuls fold the stats across partitions and expand
the per-channel affine (a, b) back to partitions; a single fused
tensor_scalar (y*a + b) and a contiguous DMA store finish each tile.

The host packs x into the exact SBUF tile layout so every DMA is fully
contiguous (8 KB per partition per transfer).
"""

import numpy as np

import concourse.bass as bass
import concourse.bacc as bacc
import concourse.tile as tile
from concourse import mybir
from concourse.bass_utils import run_bass_kernel_spmd

# ---------------------------------------------------------------- constants
B, C, H, W = 256, 128, 32, 32
NCORES = 8
CLOC = C // NCORES          # channels per core = 16
BN_EPS = 1e-5
NEG_SLOPE = 0.2
F32 = mybir.dt.float32
# bf16 x/W for the projection matmul: the PE runs bf16 at 1 cycle/row vs 4
# for fp32 (78us -> 20us of PE time), and the fp32->bf16 cast rides the
# SWDGE load DMA for free, halving the load-side DMA-engine occupancy.
# Accumulation stays fp32 in PSUM; end-to-end rel err ~3e-3 vs the 2e-2
# harness tolerance.
BF16 = mybir.dt.bfloat16


class Cfg:
    """Geometry of the per-core kernel (parametrized so a mini version can
    run through the interpreter)."""

    def __init__(self, G=4, NJG=4, TS=2048):
        self.G = G                    # channel groups (4 channels each)
        self.NJG = NJG                # DMA tiles per group
        self.TS = TS                  # free-dim columns per tile
        self.NQ = TS // 512           # matmuls (512-col chunks) per tile
        self.NB = NJG * self.NQ * 16  # batches covered (16 batches per 512 cols)
        self.NFREE = NJG * TS         # free elements per partition per group
        self.NTOT = 32 * self.NFREE   # BN element count per channel (32 k-rows)


FULL = Cfg()
assert FULL.NB == B and FULL.G * 4 == CLOC


# ------------------------------------------------------------- bass program
def build_nc(cfg: Cfg, reps: int = 1, mode: str = "full"):
    G, NJG, TS, NQ = cfg.G, cfg.NJG, cfg.TS, cfg.NQ
    # Bacc (not raw Bass): its compile() runs generate_event_semaphores,
    # which legalizes to the TRN2 1-sync-wait-per-instruction constraint.
    # Bigger SWDGE descriptor ring (1536 descs = 12 in-flight 128-desc
    # loads) so load descriptor generation isn't ring-throttled to transfer
    # pace — the partition reduces queued behind the gens on the Pool
    # engine then run ~7us earlier.
    nc = bacc.Bacc("TRN2", target_bir_lowering=False, debug=False,
                   dynamic_dma_scratch_size=24576)

    xt = nc.dram_tensor("xt", [G, NJG, 128, TS], F32, kind="ExternalInput")
    wt = nc.dram_tensor("wt", [128, G * 128], BF16, kind="ExternalInput")
    ct = nc.dram_tensor("ct", [128, const_cols(cfg)], F32, kind="ExternalInput")
    # bf16 output: halves store-side HBM traffic (the kernel is HBM-bound;
    # 16.8 MB load + 8.4 MB store = 25.2 MB/core vs 33.6 fp32). The host
    # unpack upcasts to fp32; bf16 output quantization adds ~1e-3 rel err
    # against the 2e-2 harness tolerance.
    yt = nc.dram_tensor("yt", [G, NJG, 128, TS], BF16, kind="ExternalOutput")

    with tile.TileContext(nc) as tc:
        _body(tc, {"yt": yt.ap()},
              {"xt": xt.ap(), "wt": wt.ap(), "ct": ct.ap()},
              cfg, reps=reps, mode=mode)
    nc.compile()
    return nc


def _const_offsets(cfg: Cfg):
    """Column offsets inside the packed constants panel [128, NCOLS]:
    lam | zero | gb(per-partition expanded gamma/beta, 2G cols) | eps.
    (The block-diagonal weights travel separately as bf16.)"""
    G = cfg.G
    o = {}
    o["lam"] = 0
    o["zero"] = o["lam"] + 1
    o["gb"] = o["zero"] + 1
    o["eps"] = o["gb"] + 2 * G
    o["end"] = o["eps"] + 1
    return o


def const_cols(cfg: Cfg):
    return _const_offsets(cfg)["end"]


def _body(tc, outs, ins, cfg: Cfg, reps: int = 1, mode: str = "full"):
    """Kernel body over DRAM APs (shared by the HW path and the interp test).
    reps > 1 wraps the whole body in a hardware For_i loop — used only by the
    timing bench to amplify device time above the dispatch-noise floor.
    mode: "full" = real kernel; "dmaonly" = just the load + store streams
    (garbage output) to measure the DMA roofline of this access pattern."""
    nc = tc.nc
    G, NJG, TS, NQ = cfg.G, cfg.NJG, cfg.TS, cfg.NQ
    xt, wt, ct = ins["xt"], ins["wt"], ins["ct"]
    yt = outs["yt"]
    off = _const_offsets(cfg)

    from contextlib import ExitStack
    with ExitStack() as ctx:
        singles = ctx.enter_context(tc.tile_pool(name="singles", bufs=1))
        xpool = ctx.enter_context(tc.tile_pool(name="xp", bufs=16))
        ypool = ctx.enter_context(tc.tile_pool(name="yp", bufs=1))
        # bf16 staging for normalized output: 8 bufs so the tail's 8
        # back-to-back normalize+store pairs never serialize on a staging
        # WAR (bufs=3 paced the tail at one store per 1.6us).
        spool = ctx.enter_context(tc.tile_pool(name="st", bufs=8))

        # two-bank [128, 2, 512] psum tiles; 3 bufs (+1 absorber bank) = 7 of
        # the 8 banks. Each tile takes two matmuls (one per bank) and ONE
        # [128,1024] Prelu drain: ACT carries ~370ns fixed overhead per
        # Activation, so halving the instruction count cuts ACT busy from
        # ~56us to ~33us/rep.
        pspool = ctx.enter_context(tc.tile_pool(name="ps", bufs=3, space="PSUM"))
        # Scratch PSUM bank for "wait absorber" matmuls: walrus allows only a
        # single sync-wait on a Matmult (it lands on the LDWEIGHTS half), so
        # before each tile's real matmuls a dummy 1x1 matmul absorbs the
        # x-DMA semaphore wait into PE's vector clock; the real matmuls then
        # only ever carry the one PSUM-WAR wait.
        absp = ctx.enter_context(tc.tile_pool(name="absp", bufs=1, space="PSUM"))
        abs_ps = absp.tile([1, 1], F32, tag="abs", name="abs_ps")

        if mode in ("dmaonly", "loadonly"):
            if reps > 1:
                ctx.enter_context(tc.For_i(0, reps, 1, staggered_reset=True))
            src = singles.tile([128, TS], BF16, tag="dsrc", name="dsrc")
            nc.vector.memset(src[:, 0:1], 0.0)
            for g in range(G):
                for jg in range(NJG):
                    xtile = xpool.tile([128, TS], BF16, tag="x", name=f"dx_{g}_{jg}")
                    nc.gpsimd.dma_start(out=xtile, in_=xt[g, jg])
                    if mode == "dmaonly":
                        nc.sync.dma_start(out=yt[g, jg], in_=src)
            return

        # Small constant DMAs: the bf16 block-diag weights, then the fp32
        # panel (lam broadcast, a zero column, the two selector matrices,
        # gamma/beta and eps).
        w_sb = singles.tile([128, G * 128], BF16, name="w_sb")
        nc.sync.dma_start(out=w_sb, in_=wt)
        c_sb = singles.tile([128, off["end"]], F32)
        nc.sync.dma_start(out=c_sb, in_=ct)
        lam_sb = c_sb[:, off["lam"]:off["lam"] + 1]
        zero_sb = c_sb[:, off["zero"]:off["zero"] + 1]
        gb_sb = c_sb[:, off["gb"]:off["gb"] + 2 * G]
        eps_sb = c_sb[:, off["eps"]:off["eps"] + 1]
        # ACT warmup: observe the const-DMA semaphore once so the per-tile
        # Prelu activations only ever carry the single PE sync-wait.
        act_warm = singles.tile([128, 1], F32)
        nc.scalar.activation(out=act_warm, in_=zero_sb,
                             func=mybir.ActivationFunctionType.Identity,
                             bias=zero_sb, scale=1.0)
        # PE warmup: observe the W-DMA semaphore once so the per-tile real
        # matmuls never need a second wait.
        nc.tensor.matmul(abs_ps, w_sb[0:1, 0:1], w_sb[0:1, 0:1],
                         start=True, stop=True)

        # bn_stats is HW-limited to 512 free elements, so one entry per
        # 512-col chunk (BNStats also has no 16-bit DVE fast mode).
        stats = singles.tile([128, G, NJG * NQ, 6], F32)

        # x loads ride the Pool/SWDGE queue. With bufs=16 every tile has its
        # own buffer (x is fully SBUF-resident as bf16), so no load ever
        # waits on compute and the load half of the DMA stream runs
        # back-to-back from t~2us. Only the first 3 groups' loads go
        # up-front: the SWDGE descriptor ring holds ~8 DMAs, so generation
        # runs at transfer pace and the in-order Pool queue would otherwise
        # park group 0's partition_all_reduce behind all 16 gens (~26us).
        # The last group's loads are emitted after g0's fold stage 0 instead.
        xtiles = {}
        ytiles = {}

        def load_x(g, jg):
            # gpsimd (SWDGE) so the DMA itself casts fp32 -> bf16: halves
            # the load-side DMA-engine occupancy and feeds the PE its bf16
            # operand with no extra compute pass.
            xtile = xpool.tile([128, TS], BF16, tag="x", name=f"x_{g}_{jg}")
            nc.gpsimd.dma_start(out=xtile, in_=xt[g, jg])
            xtiles[g, jg] = xtile

        # Per-group stats fold, cut into stages that are emitted interleaved
        # with the NEXT group's tile blocks (software pipeline). Every engine
        # queue is in-order, so each stage must reach the head of its queue
        # only after its inputs are ready. The fold never touches PE: the
        # across-partition reduction runs per 32-partition block on the
        # otherwise-idle Pool/GPSIMD engine, and the rest of the chain stays
        # 128 partitions wide (gamma/beta arrive host-expanded per
        # partition), so no expansion matmul is needed. Nothing ever
        # head-of-line blocks a saturated engine.
        fs = {}
        # si tiles are [128, 32] so the 32x32 stream transpose can fold
        # them; cols 2..31 are zeroed once so the transpose/reduce of the
        # unused columns never sees uninitialized SBUF.
        for f0 in range(G):
            si0 = singles.tile([128, 32], F32, tag=f"si{f0}", name=f"si_{f0}")
            nc.vector.memset(si0, 0.0)
            fs[f0] = {"si": si0}

        # The timing loop starts here: constants, warmups and memsets above
        # are genuinely one-time costs (in the single-shot kernel they hide
        # under the first load's DGE latency), so the slope bench measures
        # the steady-state pipeline only.
        # unroll=1: measured on HW, two body copies per iteration pipelined
        # WORSE than the staggered-reset back-edge (109us vs 77us per rep) —
        # the scheduler overlaps consecutive loop iterations better than
        # intra-body copies.
        unroll = int(mode[6:]) if mode.startswith("unroll") else 1
        if reps > 1:
            assert reps % unroll == 0
            ctx.enter_context(tc.For_i(0, reps // unroll, 1,
                                       staggered_reset=True))

        def emit_once():
            for g in range(G - 1):
                for jg in range(NJG):
                    load_x(g, jg)
            emit_main()

        def fold_stage(f, stage):
            if stage == 0:
                # per-partition mean/var -> scaled sum & sum-of-squares
                # (emitted right after group f's own bn_stats: DVE reaches
                # these with data already in hand); then fold across each
                # channel's 32 partitions entirely on DVE via two 32x32
                # stream transposes: transpose + free-dim reduce puts each
                # channel's (S, SS) on partitions 32c+{0,1}; broadcast-copy
                # + transpose replicates them to every partition of the
                # block. (gpsimd.partition_all_reduce silently corrupts
                # base-partition!=0 blocks on hardware.)
                mv = singles.tile([128, 2], F32, tag=f"mv{f}", name=f"mv_{f}")
                nc.vector.bn_aggr(out=mv, in_=stats[:, f, :, :])
                si = fs[f]["si"]
                nc.vector.tensor_copy(si[:, 0:1], mv[:, 0:1])
                nc.vector.tensor_mul(si[:, 1:2], mv[:, 0:1], mv[:, 0:1])
                nc.vector.tensor_add(si[:, 1:2], si[:, 1:2], mv[:, 1:2])
                nc.vector.tensor_scalar_mul(si[:, 0:2], si[:, 0:2],
                                            float(cfg.NFREE))
                T = singles.tile([128, 32], F32, tag=f"T{f}", name=f"T_{f}")
                nc.vector.transpose(T, si)
                red = singles.tile([128, 1], F32, tag=f"red{f}", name=f"red_{f}")
                nc.vector.reduce_sum(red, T, axis=mybir.AxisListType.X)
                U = singles.tile([128, 32], F32, tag=f"U{f}", name=f"U_{f}")
                nc.vector.tensor_copy(U, red[:, 0:1].to_broadcast((128, 32)))
                V = singles.tile([128, 32], F32, tag=f"V{f}", name=f"V_{f}")
                nc.vector.transpose(V, U)
                fs[f]["sAR"] = V[:, 0:2]
            elif stage == 1:
                # per-partition scalar math on DVE — every partition holds
                # its own channel's stats, so everything stays 128 wide.
                chan = singles.tile([128, 2], F32, tag=f"chan{f}",
                                    name=f"chan_{f}")
                nc.vector.tensor_scalar_mul(chan, fs[f]["sAR"],
                                            1.0 / float(cfg.NTOT))
                var1 = singles.tile([128, 1], F32, tag=f"var{f}", name=f"var_{f}")
                nc.vector.tensor_mul(var1, chan[:, 0:1], chan[:, 0:1])
                nc.vector.tensor_sub(var1, chan[:, 1:2], var1)
                fs[f]["chan"] = chan
                fs[f]["var1"] = var1
            elif stage == 2:
                # sqrt(var + eps) on ACT — same act table set as Prelu, so
                # no table reload.
                nc.scalar.activation(out=fs[f]["var1"], in_=fs[f]["var1"],
                                     func=mybir.ActivationFunctionType.Sqrt,
                                     bias=eps_sb[:, :], scale=1.0)
            elif stage == 3:
                chan, var1 = fs[f]["chan"], fs[f]["var1"]
                nc.vector.reciprocal(var1, var1)   # 1/sqrt(var+eps)
                ab = singles.tile([128, 2], F32, tag=f"ab{f}", name=f"ab_{f}")
                nc.vector.tensor_mul(ab[:, 0:1], gb_sb[:, f:f + 1], var1)
                nc.vector.tensor_mul(ab[:, 1:2], chan[:, 0:1], ab[:, 0:1])
                nc.vector.tensor_sub(ab[:, 1:2], gb_sb[:, G + f:G + f + 1],
                                     ab[:, 1:2])
                fs[f]["ab"] = ab
            else:
                raise AssertionError("stage 4 replaced by norm_store")

        def norm_store(f, jg):
            # normalize (bf16 -> bf16 on DVE: TensorScalarPtr with all-SBUF
            # 2-byte packed operands hits the 4x_2p perf mode, ~0.6us per
            # 2048-col tile) into a staging tile, then store.
            ab = fs[f]["ab"]
            ytile = ytiles[f, jg]
            stile = spool.tile([128, NQ, 512], BF16, tag="st",
                               name=f"st_{f}_{jg}")
            nc.vector.tensor_scalar(
                out=stile, in0=ytile,
                scalar1=ab[:, 0:1], scalar2=ab[:, 1:2],
                op0=mybir.AluOpType.mult, op1=mybir.AluOpType.add)
            # stores go out on the SP HWDGE queue: the casting loads own
            # GPSIMD's SWDGE queue, and a store waiting on this group's
            # normalize must not head-of-line-block loads.
            if mode != "nostore":
                nc.sync.dma_start(out=yt[f, jg], in_=stile)

        def emit_main():
            for g in range(G):
                for jg in range(NJG):
                    xtile = xtiles[g, jg]
                    # bf16 y tile: bn_stats runs at the DVE 16-bit 2x rate and
                    # SBUF stays light; the bf16 rounding happens before the
                    # batch stats, so stats and normalize see the same values.
                    ytile = ypool.tile([128, NQ, 512], BF16, tag=f"y_{g}_{jg}",
                                       name=f"y_{g}_{jg}")
                    ytiles[g, jg] = ytile
                    nc.tensor.matmul(abs_ps, xtile[0:1, 0:1], xtile[0:1, 0:1],
                                     start=True, stop=True)
                    for h in range(NQ // 2):
                        ps = pspool.tile([128, 2, 512], F32, tag="mm",
                                         name=f"mm_{g}_{jg}_{h}")
                        for j in range(2):
                            q = 2 * h + j
                            nc.tensor.matmul(ps[:, j, :],
                                             w_sb[:, g * 128:(g + 1) * 128],
                                             xtile[:, q * 512:(q + 1) * 512],
                                             start=True, stop=True)
                        # NOTE: Prelu, not Lrelu — the HW Lrelu table ignores the
                        # alpha operand (fixed 0.01 slope); Prelu honors it.
                        # lam is folded into the weights on the host (BN is
                        # scale-invariant up to the eps'=eps/lam^2 correction
                        # packed into the constants panel): scale=1.
                        nc.scalar.activation(
                            out=ytile[:, 2 * h:2 * h + 2, :], in_=ps,
                            func=mybir.ActivationFunctionType.Prelu,
                            bias=zero_sb[:, :], scale=1.0, alpha=NEG_SLOPE)
                        # bn_stats right behind each Prelu (HW caps bn_stats
                        # at 512 free elements, so two per 1024-col drain):
                        # keeps the last tile's stats chain short.
                        for j in range(2):
                            q = 2 * h + j
                            nc.vector.bn_stats(out=stats[:, g, jg * NQ + q, :],
                                               in_=ytile[:, q, :])
                    if mode == "nofold":
                        # diagnostic: store the un-normalized bf16 y directly,
                        # skipping the stats fold + normalize stages.
                        nc.sync.dma_start(out=yt[g, jg], in_=ytile)
                        if g == 1 and jg == NJG - 1:
                            for jg2 in range(NJG):
                                load_x(G - 1, jg2)
                        continue
                    if g >= 1 and jg == 0:
                        # whole fold + normalize burst of the previous group
                        # right after this group's first tile: spreading the
                        # normalizes one-per-tile-slot instead measured WORSE
                        # (92.0 vs 88.3us) — a late ab value head-of-line
                        # blocks DVE behind each spread normalize.
                        for stage in range(1, 4):
                            fold_stage(g - 1, stage)
                        for jg2 in range(NJG):
                            norm_store(g - 1, jg2)
                    if jg == NJG - 1:
                        fold_stage(g, 0)
                        if g == 1:
                            # last group's loads, queued behind g0's AND g1's
                            # partition reduces on the Pool queue: both early
                            # folds clear the queue before these four gens, and
                            # the transfers backfill the DMA engines behind the
                            # first stores.
                            for jg2 in range(NJG):
                                load_x(G - 1, jg2)
            if mode != "nofold":
                for stage in range(1, 4):
                    fold_stage(G - 1, stage)
                for jg in range(NJG):
                    norm_store(G - 1, jg)

        for _u in range(unroll):
            emit_once()


# ------------------------------------------------------------ host packing
def _pack_x_shard(xs, cfg: Cfg):
    """xs [NB, 4G, 32, 32] -> [G, NJG, 128, TS] tile layout.
    partition = 32*i + h ; col = jj*512 + bl*32 + w ; b = jg*(NQ*16) + jj*16 + bl."""
    G, NJG, NQ, TS = cfg.G, cfg.NJG, cfg.NQ, cfg.TS
    t = xs.reshape(NJG, NQ, 16, G, 4, H, W)          # [jg, jj, bl, g, i, h, w]
    t = t.transpose(3, 0, 4, 5, 1, 2, 6)             # [g, jg, i, h, jj, bl, w]
    return np.ascontiguousarray(t).reshape(G, NJG, 128, TS)


def _unpack_y_shard(ytv, cfg: Cfg):
    """[G, NJG, 128, TS] -> [NB, 4G, 32, 32]."""
    G, NJG, NQ, TS = cfg.G, cfg.NJG, cfg.NQ, cfg.TS
    t = ytv.reshape(G, NJG, 4, 32, NQ, 16, W)        # [g, jg, i, k, jj, bl, w]
    t = t.transpose(1, 4, 5, 0, 2, 3, 6)             # [jg, jj, bl, g, i, k, w]
    return t.reshape(cfg.NB, 4 * G, H, W)


def _pack_w(Pshard, cfg: Cfg, sgn=1.0):
    """Block-diagonal bf16 weight panel [128, G*128]: per group g four
    diagonal 32x32 blocks, each sgn*P[4g+i].T. BatchNorm is invariant to a
    positive scale on its input, so only sign(lam) must reach the kernel —
    |lam| is folded away entirely and the device never sees lam."""
    import ml_dtypes
    G = cfg.G
    w = np.zeros((128, G * 128), np.float32)
    for g in range(G):
        for i in range(4):
            w[32 * i:32 * (i + 1),
              g * 128 + 32 * i:g * 128 + 32 * (i + 1)] = Pshard[4 * g + i].T
    return (np.float32(sgn) * w).astype(ml_dtypes.bfloat16)


def _pack_const(Pshard, lam, gamma_s, beta_s, cfg: Cfg):
    """Pack the small fp32 constants into one [128, NCOLS] panel.
    gamma/beta are pre-expanded per partition: col g holds
    gamma[4g + p//32] at partition p (the fold chain stays 128 wide)."""
    G = cfg.G
    off = _const_offsets(cfg)
    c = np.zeros((128, off["end"]), np.float32)
    c[:, off["lam"]] = np.float32(lam[0])
    # off["zero"] column stays 0
    blk = np.arange(128) // 32                      # channel-in-group index
    for g in range(G):
        c[:, off["gb"] + g] = gamma_s[4 * g + blk]
        c[:, off["gb"] + G + g] = beta_s[4 * g + blk]
    # The kernel computes stats of u = leaky(sign(lam)*proj), i.e. y/|lam|.
    # Exactly: (y-mean_y)*rsqrt(var_y+eps) == (u-mean_u)*rsqrt(var_u+eps/lam^2),
    # so the eps the kernel adds must be pre-divided by lam^2.
    lam2 = float(lam[0]) ** 2
    c[:, off["eps"]] = BN_EPS / lam2 if lam2 > 0 else BN_EPS
    return c


def make_in_maps(x, P, lam, gamma, beta, cfg: Cfg = FULL, ncores: int = NCORES):
    cl = 4 * cfg.G
    sgn = 1.0 if float(lam[0]) >= 0 else -1.0
    maps = []
    for m in range(ncores):
        sl = slice(m * cl, (m + 1) * cl)
        maps.append({
            "xt": _pack_x_shard(np.ascontiguousarray(x[:, sl]), cfg),
            "wt": _pack_w(P[sl], cfg, sgn),
            "ct": _pack_const(P[sl], lam, gamma[sl], beta[sl], cfg),
        })
    return maps


_NC_CACHE = {}


def _get_nc(cfg: Cfg = FULL):
    key = (cfg.G, cfg.NJG, cfg.TS)
    if key not in _NC_CACHE:
        _NC_CACHE[key] = build_nc(cfg)
    return _NC_CACHE[key]


def run(inputs, trace=False, tmpdir=None):
    """Run on the 8 NeuronCores; returns (out, BassKernelResults)."""
    x = np.asarray(inputs["x"], np.float32)
    P = np.asarray(inputs["P"], np.float32)
    lam = np.asarray(inputs["lam"], np.float32)
    gamma = np.asarray(inputs["gamma"], np.float32)
    beta = np.asarray(inputs["beta"], np.float32)

    if float(lam[0]) == 0.0:
        # y == 0 everywhere -> BN emits exactly beta (matches reference).
        out = np.broadcast_to(beta[None, :, None, None],
                              (B, C, H, W)).astype(np.float32).copy()
        return out, None

    nc = _get_nc(FULL)
    in_maps = make_in_maps(x, P, lam, gamma, beta, FULL)
    res = run_bass_kernel_spmd(nc, in_maps, core_ids=list(range(NCORES)),
                               trace=trace, tmpdir=tmpdir)
    out = np.empty((B, C, H, W), np.float32)
    for m in range(NCORES):
        out[:, m * CLOC:(m + 1) * CLOC] = _unpack_y_shard(
            np.asarray(res.results[m]["yt"]).astype(np.float32), FULL)
    return out, res


def kernel(**inputs):
    out, _ = run(inputs)
    return out

